# revision 1
# baseline (speedup 1.0000x reference)
"""HGCN (HAN-style) layer on 8 trn2 NeuronCores.

Strategy (per sharding hint): shard dst nodes across cores (6250/core),
partition edges by dst on host, add self-loops, precompute GCN norm
scalars. On device, per 128-dst block: indirect-DMA gather fp16 source
rows (128 edges/tile), build a norm-valued one-hot [edge, dstoff] tile,
matmul-scatter into PSUM (accumulating over tiles), then z = agg @ W,
semantic attention over the 4 meta-paths, weighted blend, DMA out.
No collectives needed: each core reads the replicated x table.
"""
import os
import sys

for _p in ("/opt/trn_rl_repo", "/opt/pypackages"):
    if _p not in sys.path and os.path.isdir(_p):
        sys.path.insert(0, _p)

import numpy as np

N_NODES = 50000
N_PATHS = 4
D = 256
D_HID = 128
N_CORES = 8
NPC = N_NODES // N_CORES  # 6250
BLK = 128
NBLK = (NPC + BLK - 1) // BLK  # 49
TILE_E = 128

GATHER_DTYPE = np.float16  # dtype of the gathered x table / scatter matmul

LAST_RESULTS = None  # BassKernelResults of the most recent run (for test.py)


# ---------------------------------------------------------------- host prep
def _preprocess(x, edge_indices):
    E = np.asarray(edge_indices)
    dinv = np.empty((N_PATHS, N_NODES), np.float32)
    for p in range(N_PATHS):
        deg = np.bincount(E[p, 1], minlength=N_NODES).astype(np.float32) + 1.0
        dinv[p] = 1.0 / np.sqrt(deg)

    segs = {}
    Treq = np.zeros((N_CORES, N_PATHS, NBLK), np.int64)
    for c in range(N_CORES):
        lo, hi = c * NPC, (c + 1) * NPC
        for p in range(N_PATHS):
            src, dst = E[p, 0], E[p, 1]
            m = (dst >= lo) & (dst < hi)
            s_ = src[m].astype(np.int64)
            d_ = dst[m].astype(np.int64)
            n_ = dinv[p][s_] * dinv[p][d_]
            loop = np.arange(lo, hi, dtype=np.int64)
            s_ = np.concatenate([s_, loop])
            d_ = np.concatenate([d_, loop])
            n_ = np.concatenate([n_, dinv[p][loop] ** 2]).astype(np.float32)
            rel = d_ - lo
            blk = rel >> 7
            order = np.argsort(blk, kind="stable")
            s_, rel, n_ = s_[order], rel[order], n_[order]
            cnt = np.bincount(rel >> 7, minlength=NBLK)
            segs[(c, p)] = (s_, (rel & 127), n_, cnt)
            Treq[c, p] = (cnt + TILE_E - 1) // TILE_E

    T = Treq.max(axis=0).T.copy()  # [NBLK, N_PATHS]
    Ttot = int(T.sum())

    per_core = []
    for c in range(N_CORES):
        srcT = np.zeros((TILE_E, Ttot), np.int32)
        dofT = np.zeros((TILE_E, Ttot), np.float32)
        nrmT = np.zeros((TILE_E, Ttot), np.float32)
        pos = {p: 0 for p in range(N_PATHS)}
        col = 0
        for b in range(NBLK):
            for p in range(N_PATHS):
                s_, doff, n_, cnt = segs[(c, p)]
                k = int(cnt[b])
                t = int(T[b][p])
                sl = slice(pos[p], pos[p] + k)
                pos[p] += k
                for buf, src_arr, dt in ((srcT, s_, np.int32),
                                         (dofT, doff, np.float32),
                                         (nrmT, n_, np.float32)):
                    tmp = np.zeros(t * TILE_E, dt)
                    tmp[:k] = src_arr[sl]
                    buf[:, col:col + t] = tmp.reshape(t, TILE_E).T
                col += t
        # dof in columns [0:Ttot], nrm in [Ttot:2Ttot] — single DMA on device
        meta = np.concatenate([dofT, nrmT], axis=1)
        per_core.append(dict(srcT=srcT, meta=meta))

    x_g = np.ascontiguousarray(np.asarray(x).astype(GATHER_DTYPE))
    return x_g, per_core, T


def _pack_consts(Ws, attn_w1, attn_w2):
    """One [128, NCONST] f32 array: Ws (4 paths x 2 k-chunks x 256 cols),
    w1 (2 k-chunks x 128), w2 (1), iota (128), identity (128)."""
    cols = []
    for p in range(N_PATHS):
        for h in range(2):
            cols.append(np.asarray(Ws[p, h * 128:(h + 1) * 128, :],
                                   np.float32))  # [128, 256]
    for h in range(2):
        cols.append(np.asarray(attn_w1[h * 128:(h + 1) * 128, :], np.float32))
    cols.append(np.asarray(attn_w2, np.float32).reshape(128, 1))
    cols.append(np.tile(np.arange(BLK, dtype=np.float32)[None, :], (BLK, 1)))
    cols.append(np.eye(BLK, dtype=np.float32))
    return np.ascontiguousarray(np.concatenate(cols, axis=1))


# ---------------------------------------------------------------- device prog
def _build(T, xg_np_dtype):
    from concourse import bacc, bass, mybir, tile
    from concourse.bass import IndirectOffsetOnAxis
    from concourse.masks import make_identity

    f32 = mybir.dt.float32
    f16 = mybir.dt.float16 if xg_np_dtype == np.float16 else mybir.dt.bfloat16
    i32 = mybir.dt.int32
    Ttot = int(T.sum())

    # consts layout (columns): 4 paths x 2 chunks x 256 (Ws), 2 x 128 (w1),
    # 1 (w2), 128 (iota), 128 (identity)
    NCONST = N_PATHS * 2 * D + 2 * BLK + 1 + BLK + BLK

    nc = bacc.Bacc()
    xg_d = nc.declare_dram_parameter("xg", [N_NODES, D], f16, isOutput=False)
    srcT_d = nc.declare_dram_parameter("srcT", [TILE_E, Ttot], i32, isOutput=False)
    meta_d = nc.declare_dram_parameter("meta", [TILE_E, 2 * Ttot], f32,
                                       isOutput=False)
    consts_d = nc.declare_dram_parameter("consts", [BLK, NCONST], f32,
                                         isOutput=False)
    out_d = nc.declare_dram_parameter("out", [NBLK * BLK, D], f32, isOutput=True)

    AluOp = mybir.AluOpType
    Act = mybir.ActivationFunctionType

    with tile.TileContext(nc) as tc:
        with (
            tc.tile_pool(name="const", bufs=1) as cpool,
            tc.tile_pool(name="edges", bufs=1) as epool,
            tc.tile_pool(name="xsrc", bufs=8) as xpool,
            tc.tile_pool(name="sh", bufs=8) as shpool,
            tc.tile_pool(name="work", bufs=2) as wpool,
            tc.tile_pool(name="zbuf", bufs=6) as zpool,
            tc.tile_pool(name="outb", bufs=3) as opool,
            tc.tile_pool(name="agg_ps", bufs=2, space="PSUM") as agg_pp,
            tc.tile_pool(name="tr_ps", bufs=2, space="PSUM") as tr_pp,
            tc.tile_pool(name="z_ps", bufs=2, space="PSUM") as z_pp,
            tc.tile_pool(name="h_ps", bufs=1, space="PSUM") as h_pp,
            tc.tile_pool(name="s_ps", bufs=1, space="PSUM") as s_pp,
        ):
            # constants: one packed DMA; slice views into the packed tile
            consts_sb = cpool.tile([BLK, NCONST], f32, tag="consts")
            nc.sync.dma_start(out=consts_sb[:], in_=consts_d[:])
            off = 0
            W_sb = []  # W_sb[p][h] -> [128, 256] AP
            for p in range(N_PATHS):
                hs = []
                for h in range(2):
                    hs.append(consts_sb[:, off:off + D])
                    off += D
                W_sb.append(hs)
            w1_sb = []
            for h in range(2):
                w1_sb.append(consts_sb[:, off:off + BLK])
                off += BLK
            w2_sb = consts_sb[:, off:off + 1]
            off += 1
            iota_f = consts_sb[:, off:off + BLK]
            off += BLK
            ident = consts_sb[:, off:off + BLK]
            off += BLK
            assert off == NCONST
            # edge metadata, resident in SBUF
            srcT_sb = epool.tile([TILE_E, Ttot], i32, tag="srcT")
            nc.sync.dma_start(out=srcT_sb[:], in_=srcT_d[:])
            meta_sb = epool.tile([TILE_E, 2 * Ttot], f32, tag="meta")
            nc.sync.dma_start(out=meta_sb[:], in_=meta_d[:])

            col = 0
            for b in range(NBLK):
                z_tiles = []
                s_ps = s_pp.tile([BLK, N_PATHS], f32, tag="s")
                for p in range(N_PATHS):
                    t = int(T[b][p])
                    agg_ps = agg_pp.tile([BLK, D], f32, tag="agg")
                    for ti in range(t):
                        xs = xpool.tile([BLK, D], f16, tag="xs")
                        nc.gpsimd.indirect_dma_start(
                            out=xs[:], out_offset=None, in_=xg_d[:],
                            in_offset=IndirectOffsetOnAxis(
                                ap=srcT_sb[:, col + ti:col + ti + 1], axis=0))
                        sh = shpool.tile([BLK, BLK], f16, tag="sh")
                        nc.vector.tensor_scalar(
                            out=sh[:], in0=iota_f,
                            scalar1=meta_sb[:, col + ti:col + ti + 1],
                            scalar2=meta_sb[:, Ttot + col + ti:
                                            Ttot + col + ti + 1],
                            op0=AluOp.is_equal, op1=AluOp.mult)
                        nc.tensor.matmul(out=agg_ps[:], lhsT=sh[:], rhs=xs[:],
                                         start=(ti == 0), stop=(ti == t - 1))
                    col += t
                    agg_sb = wpool.tile([BLK, D], f32, tag="agg_sb")
                    nc.scalar.activation(out=agg_sb[:], in_=agg_ps[:],
                                         func=Act.Copy)
                    trp = tr_pp.tile([BLK, D], f32, tag="tr")
                    nc.tensor.transpose(out=trp[:, 0:128],
                                        in_=agg_sb[:, 0:128], identity=ident)
                    nc.tensor.transpose(out=trp[:, 128:256],
                                        in_=agg_sb[:, 128:256], identity=ident)
                    aggT_sb = wpool.tile([BLK, D], f32, tag="aggT_sb")
                    nc.scalar.activation(out=aggT_sb[:], in_=trp[:],
                                         func=Act.Copy)
                    z_ps = z_pp.tile([BLK, D], f32, tag="z")
                    nc.tensor.matmul(out=z_ps[:], lhsT=aggT_sb[:, 0:128],
                                     rhs=W_sb[p][0], start=True, stop=False)
                    nc.tensor.matmul(out=z_ps[:], lhsT=aggT_sb[:, 128:256],
                                     rhs=W_sb[p][1], start=False, stop=True)
                    z_sb = zpool.tile([BLK, D], f32, tag="z_sb")
                    nc.scalar.activation(out=z_sb[:], in_=z_ps[:], func=Act.Copy)
                    trp2 = tr_pp.tile([BLK, D], f32, tag="tr")
                    nc.tensor.transpose(out=trp2[:, 0:128],
                                        in_=z_sb[:, 0:128], identity=ident)
                    nc.tensor.transpose(out=trp2[:, 128:256],
                                        in_=z_sb[:, 128:256], identity=ident)
                    zT_sb = wpool.tile([BLK, D], f32, tag="zT_sb")
                    nc.scalar.activation(out=zT_sb[:], in_=trp2[:], func=Act.Copy)
                    h_ps = h_pp.tile([BLK, D_HID], f32, tag="h")
                    nc.tensor.matmul(out=h_ps[:], lhsT=w1_sb[0],
                                     rhs=zT_sb[:, 0:128], start=True, stop=False)
                    nc.tensor.matmul(out=h_ps[:], lhsT=w1_sb[1],
                                     rhs=zT_sb[:, 128:256], start=False, stop=True)
                    h_sb = wpool.tile([BLK, D_HID], f32, tag="h_sb")
                    nc.scalar.activation(out=h_sb[:], in_=h_ps[:], func=Act.Tanh)
                    nc.tensor.matmul(out=s_ps[:, p:p + 1], lhsT=h_sb[:],
                                     rhs=w2_sb, start=True, stop=True)
                    z_tiles.append(z_sb)
                # semantic attention (softmax over the 4 paths, no max-shift:
                # |s| <= sum|w2| so exp stays well within fp32 range)
                e_sb = wpool.tile([BLK, N_PATHS], f32, tag="e")
                nc.scalar.activation(out=e_sb[:], in_=s_ps[:], func=Act.Exp)
                den = wpool.tile([BLK, 1], f32, tag="den")
                nc.vector.tensor_reduce(out=den[:], in_=e_sb[:],
                                        axis=mybir.AxisListType.X,
                                        op=AluOp.add)
                rden = wpool.tile([BLK, 1], f32, tag="rden")
                nc.vector.reciprocal(out=rden[:], in_=den[:])
                beta = wpool.tile([BLK, N_PATHS], f32, tag="beta")
                nc.vector.tensor_scalar_mul(out=beta[:], in0=e_sb[:],
                                            scalar1=rden[:, 0:1])
                out_sb = opool.tile([BLK, D], f32, tag="out_sb")
                nc.scalar.activation(out=out_sb[:], in_=z_tiles[0][:],
                                     func=Act.Copy, scale=beta[:, 0:1])
                for p in range(1, N_PATHS):
                    tmp = opool.tile([BLK, D], f32, tag="tmp")
                    nc.scalar.activation(out=tmp[:], in_=z_tiles[p][:],
                                         func=Act.Copy, scale=beta[:, p:p + 1])
                    nc.vector.tensor_add(out=out_sb[:], in0=out_sb[:],
                                         in1=tmp[:])
                nc.sync.dma_start(out=out_d[b * BLK:(b + 1) * BLK, :],
                                  in_=out_sb[:])
    nc.compile()
    return nc


# ---------------------------------------------------------------- entry point
def kernel(x, edge_indices, Ws, bs, attn_w1, attn_b1, attn_w2):
    global LAST_RESULTS
    from concourse.bass_utils import run_bass_kernel_spmd

    assert not np.any(np.asarray(bs)), "kernel assumes bs == 0"
    assert not np.any(np.asarray(attn_b1)), "kernel assumes attn_b1 == 0"

    x_g, per_core, T = _preprocess(x, edge_indices)
    nc = _build(T, GATHER_DTYPE)

    consts = _pack_consts(np.asarray(Ws), np.asarray(attn_w1),
                          np.asarray(attn_w2))
    in_maps = [
        dict(xg=x_g, srcT=pc["srcT"], meta=pc["meta"], consts=consts)
        for pc in per_core
    ]
    res = run_bass_kernel_spmd(nc, in_maps, list(range(N_CORES)))
    LAST_RESULTS = res
    out = np.concatenate([res.results[c]["out"][:NPC]
                          for c in range(N_CORES)], axis=0)
    return out.astype(np.float32)

